# revision 1
# baseline (speedup 1.0000x reference)
"""NextVLAD Trainium2 kernel v3 — 8-way data-parallel over batch (1 sample/core).

Host prep: x is token-normalized on host (exact), so every device scale is a
compile-time constant; W3 = [(W_gk@W_inp)^T | (W_g@W_inp)^T]*SW3 folds the
fc and gate projections for the softmax path; W1 = W_inp^T*SW1 feeds yT.
Only one act table (exp) is ever loaded; softmax denominators use
reciprocal_approx_fast; final L2 norms applied on host.

Per-core dataflow (sample b; M=512 tokens, N=1024, EN=2048, G=8, K=128, D=256):
  z[m,gk]   = x8^T W3-chunks (fp8 DR)   ex = exp(z/16384)      bf16
  zG[m,g]   = x8^T W3-gatecols          u  = exp(-z/16384)*e^{-bg'}
                                        sg = 1/(1+u)
  se[gk]    = ones^T ex (bf16 mm)       ise = recip_approx(se) f32
  rawY[m,e] = x8^T W1-chunks (fp8 DR)   yT = rawY/128 + binp   bf16
  wf[m,gk]  = ex * sg * ise                                    bf16
  vd[k,258] = sum_{g,m} wf_g^T [yT_g | 1 1]   (col 256 = S[k])
  out[k,d]  = vd - S*cent            (unnormalized; host l2-norms)

DMA dispatch costs ~585 ns per dma_start on every capable engine (SP, Pool,
Act), so dispatches are spread across all three, ordered by need.
"""
import os
import numpy as np

N = 1024          # feature size
EN = 2048         # expanded features
G = 8             # groups
KC = 128          # clusters
D = 256           # per-group cluster dim
GK = G * KC       # 1024
BW = D + 2        # yT group block width (data + ones + pad)
M = 512           # tokens per sample
MT = 4            # m-tiles of 128
W1C = EN          # 2048 cols in w1 (stride already % 16 == 0)
W3W = 1040        # 1024 gk + 8 gates + 8 pad
SX = 8.0
SW1 = 16.0
SW3 = 2048.0
EXS = 1.0 / (SX * SW3)    # 1/16384
YTS = 1.0 / (SX * SW1)    # 1/128

_cache = {}


def _build_nc():
    import concourse.bacc as bacc
    import concourse.tile as tile
    from concourse import mybir

    f32 = mybir.dt.float32
    bf16 = mybir.dt.bfloat16
    fp8 = mybir.dt.float8e4
    Alu = mybir.AluOpType
    Act = mybir.ActivationFunctionType
    DR = mybir.MatmulPerfMode.DoubleRow

    nc = bacc.Bacc("TRN2", target_bir_lowering=False)
    x_d = nc.dram_tensor("x", [N, M], fp8, kind="ExternalInput")
    w1_d = nc.dram_tensor("w1", [N, W1C], fp8, kind="ExternalInput")
    w3_d = nc.dram_tensor("w3", [N, W3W], fp8, kind="ExternalInput")
    binp_d = nc.dram_tensor("binp", [1, W1C], f32, kind="ExternalInput")
    crb_d = nc.dram_tensor("crb", [128, 128], bf16, kind="ExternalInput")
    cf_d = nc.dram_tensor("cf", [128, 268], f32, kind="ExternalInput")
    out_d = nc.dram_tensor("out", [KC, D], bf16, kind="ExternalOutput")

    with tile.TileContext(nc) as tc:
        with tc.tile_pool(name="const", bufs=1) as constp, \
             tc.tile_pool(name="data", bufs=1) as datap, \
             tc.tile_pool(name="work", bufs=1) as workp, \
             tc.tile_pool(name="ps", bufs=1, space="PSUM") as ps:

            # ---------------- tiles ----------------
            cf_t = constp.tile([128, 268], f32)
            crb_t = constp.tile([128, 128], bf16)
            centn_t = cf_t[:, 0:D]
            ebg_t = cf_t[:, 256:264]
            xp = [datap.tile([128, 2 * M], fp8, name=f"xp{c}") for c in range(4)]
            xpv = [t.rearrange("p (s m) -> p s m", m=M) for t in xp]
            # column-split weight tiles: consumers wait only on their own tile
            w3ap = [datap.tile([128, 2 * 512], fp8, name=f"w3a{c}") for c in range(4)]
            w3av = [t.rearrange("p (s j) -> p s j", j=512) for t in w3ap]
            w3bp = [datap.tile([128, 2 * 528], fp8, name=f"w3b{c}") for c in range(4)]
            w3bv = [t.rearrange("p (s j) -> p s j", j=528) for t in w3bp]
            w1ap = [datap.tile([128, 2 * 1024], fp8, name=f"w1a{c}") for c in range(4)]
            w1av = [t.rearrange("p (s j) -> p s j", j=1024) for t in w1ap]
            w1bp = [datap.tile([128, 2 * 1024], fp8, name=f"w1b{c}") for c in range(4)]
            w1bv = [t.rearrange("p (s j) -> p s j", j=1024) for t in w1bp]
            binp_b = constp.tile([128, W1C], f32)

            # ---------------- input DMA dispatches ----------------
            def dma_xp(eng, c, s):
                r0 = c * 256 + s * 128
                eng.dma_start(out=xpv[c][:, s, :], in_=x_d[r0:r0 + 128, :])

            def dma_w3a(eng, c, s):
                r0 = c * 256 + s * 128
                eng.dma_start(out=w3av[c][:, s, :], in_=w3_d[r0:r0 + 128, 0:512])

            def dma_w3b(eng, c, s):
                r0 = c * 256 + s * 128
                eng.dma_start(out=w3bv[c][:, s, :],
                              in_=w3_d[r0:r0 + 128, 512:1040])

            def dma_w1(eng, c, s, h):
                hv = (w1av, w1bv)[h]
                r0 = c * 256 + s * 128
                eng.dma_start(out=hv[c][:, s, :],
                              in_=w1_d[r0:r0 + 128, h * 1024:(h + 1) * 1024])

            # gpsimd dispatches start ~5 us before SP's (SP has a long
            # preamble) -> critical chunk-0/1 pieces go on gpsimd
            nc.gpsimd.dma_start(out=crb_t[:], in_=crb_d[:])
            for c in (0, 1):
                dma_xp(nc.gpsimd, c, 0); dma_xp(nc.gpsimd, c, 1)
                dma_w3a(nc.gpsimd, c, 0); dma_w3a(nc.gpsimd, c, 1)
                dma_w3b(nc.gpsimd, c, 0); dma_w3b(nc.gpsimd, c, 1)
            for c in (0, 1):
                dma_w1(nc.gpsimd, c, 0, 0); dma_w1(nc.gpsimd, c, 1, 0)

            for c in (2, 3):
                dma_xp(nc.sync, c, 0); dma_xp(nc.sync, c, 1)
                dma_w3a(nc.sync, c, 0); dma_w3a(nc.sync, c, 1)
                dma_w3b(nc.sync, c, 0); dma_w3b(nc.sync, c, 1)
            for c in (2, 3):
                dma_w1(nc.sync, c, 0, 0); dma_w1(nc.sync, c, 1, 0)

            nc.scalar.dma_start(out=cf_t[:], in_=cf_d[:])

            # persistent work tiles
            ex_t = [workp.tile([128, GK], bf16, name=f"ex{m}") for m in range(MT)]
            wf_t = [workp.tile([128, GK], bf16, name=f"wf{m}") for m in range(MT)]
            yT_t = [[workp.tile([128, 2 * BW], bf16, name=f"yT{e}_{m}")
                     for m in range(MT)] for e in range(4)]
            ise_t = workp.tile([128, GK], f32)
            sg_t = workp.tile([128, 4 * G], f32)
            dum_t = workp.tile([128, 1], f32)

            # warm the exp table early (hidden under DMA lead-in)
            nc.scalar.activation(dum_t[:], cf_t[:, 264:265], Act.Exp)
            # rest of w1 + binp dispatched on scalar after the table warm
            nc.scalar.dma_start(out=binp_b[:],
                                in_=binp_d[:].to_broadcast([128, W1C]))
            for c in range(4):
                dma_w1(nc.scalar, c, 0, 1); dma_w1(nc.scalar, c, 1, 1)
            # w3 gate columns live in w3b at cols 512:520

            # yT ones columns (static)
            for e in range(4):
                for m in range(MT):
                    yT3 = yT_t[e][m].rearrange("p (g c) -> p g c", c=BW)
                    nc.vector.memset(yT3[:, :, D:D + 2], 1.0)


            # ---------------- phase A: gk logits + exp ----------------
            # chunk-major in groups of 4 chains: a late-arriving chunk c
            # stalls the PE once per group instead of once per chain
            for m in range(MT):
                msl = slice(m * 128, (m + 1) * 128)
                for h, w3hv in enumerate((w3av, w3bv)):
                    z_ps = ps.tile([128, 512], f32, name="z_ps", tag="zps",
                                   bufs=4)
                    for c in range(4):
                        nc.tensor.matmul(z_ps[:], xpv[c][:, :, msl],
                                         w3hv[c][:, :, 0:512],
                                         start=(c == 0), stop=(c == 3),
                                         perf_mode=DR)
                    nc.scalar.activation(ex_t[m][:, h * 512:(h + 1) * 512],
                                         z_ps[:], Act.Exp, scale=EXS)

            # ---------------- phase B: gates ----------------
            for m in range(MT):
                msl = slice(m * 128, (m + 1) * 128)
                zg_ps = ps.tile([128, G], f32, name="zg_ps", tag="gps", bufs=1)
                for c in range(4):
                    nc.tensor.matmul(zg_ps[:], xpv[c][:, :, msl],
                                     w3bv[c][:, :, 512:512 + G],
                                     start=(c == 0), stop=(c == 3), perf_mode=DR)
                ug = workp.tile([128, G], f32, name="ug", bufs=2)
                nc.scalar.activation(ug[:], zg_ps[:], Act.Exp, scale=-EXS)
                vg = workp.tile([128, G], f32, name="vg", bufs=2)
                nc.vector.tensor_mul(vg[:], ug[:], ebg_t)
                nc.vector.tensor_scalar_add(vg[:], vg[:], 1.0)
                nc.vector.reciprocal(sg_t[:, m * G:(m + 1) * G], vg[:])

            # ---------------- phase C: softmax denominators ----------------
            for h in range(2):
                se_ps = ps.tile([128, 512], f32, name="se_ps", tag="zps", bufs=4)
                for m in range(MT):
                    nc.tensor.matmul(se_ps[:], crb_t[:],
                                     ex_t[m][:, h * 512:(h + 1) * 512],
                                     start=(m == 0), stop=(m == MT - 1))
                nc.vector.reciprocal_approx_fast(
                    out=ise_t[:, h * 512:(h + 1) * 512], in_=se_ps[:])

            # ---------------- phase D: yT (+ wf on DVE, vlad trailing) ----
            vd_ps = ps.tile([128, BW], f32, name="vd_ps", tag="gps", bufs=1)
            binp3 = binp_b.rearrange("p (g c) -> p g c", c=D)
            def emit_wf(g):
                gsl = slice(g * KC, (g + 1) * KC)
                for m in range(MT):
                    nc.vector.scalar_tensor_tensor(
                        out=wf_t[m][:, gsl], in0=ex_t[m][:, gsl],
                        scalar=sg_t[:, m * G + g:m * G + g + 1],
                        in1=ise_t[:, gsl], op0=Alu.mult, op1=Alu.mult)

            for ec in range(4):
                for m in range(MT):
                    msl = slice(m * 128, (m + 1) * 128)
                    ry_ps = ps.tile([128, 512], f32, name="ry_ps", tag="yps", bufs=3)
                    w1hv = (w1av, w1bv)[ec // 2]
                    ecs = (ec % 2) * 512
                    for c in range(4):
                        nc.tensor.matmul(ry_ps[:], xpv[c][:, :, msl],
                                         w1hv[c][:, :, ecs:ecs + 512],
                                         start=(c == 0), stop=(c == 3),
                                         perf_mode=DR)
                    yT3 = yT_t[ec][m].rearrange("p (g c) -> p g c", c=BW)
                    nc.vector.scalar_tensor_tensor(
                        out=yT3[:, 0:2, 0:D],
                        in0=ry_ps[:].rearrange("p (g c) -> p g c", c=D),
                        scalar=YTS,
                        in1=binp3[:, 2 * ec:2 * ec + 2, :],
                        op0=Alu.mult, op1=Alu.add)
                emit_wf(2 * ec); emit_wf(2 * ec + 1)
                # vlad groups one ec late (their wf/yT are complete by now)
                if ec >= 1:
                    for g in (2 * ec - 2, 2 * ec - 1):
                        for m in range(MT):
                            nc.tensor.matmul(
                                vd_ps[:], wf_t[m][:, g * KC:(g + 1) * KC],
                                yT_t[g // 2][m][:, (g % 2) * BW:(g % 2 + 1) * BW],
                                start=(g == 0 and m == 0), stop=False)

            # ---------------- phase E: remaining vlad + centroid ----------------
            for g in (6, 7):
                for m in range(MT):
                    nc.tensor.matmul(vd_ps[:], wf_t[m][:, g * KC:(g + 1) * KC],
                                     yT_t[g // 2][m][:, (g % 2) * BW:(g % 2 + 1) * BW],
                                     start=False, stop=(g == 7 and m == MT - 1))
            vlad_t = workp.tile([128, D], bf16)
            nc.vector.scalar_tensor_tensor(
                out=vlad_t[:], in0=centn_t[:], scalar=vd_ps[:, D:D + 1],
                in1=vd_ps[:, 0:D], op0=Alu.mult, op1=Alu.add)
            for q, eng in enumerate((nc.sync, nc.gpsimd, nc.scalar, nc.sync)):
                eng.dma_start(out=out_d[q * 32:(q + 1) * 32, :],
                              in_=vlad_t[q * 32:(q + 1) * 32, :])

    nc.compile()
    return nc


def _get_nc():
    if "nc" not in _cache:
        _cache["nc"] = _build_nc()
    return _cache["nc"]


def kernel(x, W_inp, b_inp, W_g, b_g, W_gk, b_gk, centroids):
    from concourse.bass_utils import run_bass_kernel_spmd
    import ml_dtypes
    f8 = ml_dtypes.float8_e4m3

    nc = _get_nc()

    x = np.asarray(x, dtype=np.float32)
    X = x.reshape(8, 8, N, 64).transpose(0, 2, 1, 3).reshape(8, N, M)
    # exact per-token normalization on host
    X = X / np.maximum(np.linalg.norm(X, axis=1, keepdims=True), 1e-12)
    W1 = np.ascontiguousarray(
        (np.asarray(W_inp, np.float32).T * SW1).astype(f8))
    W3f = (np.asarray(W_gk, np.float32) @ np.asarray(W_inp, np.float32)).T
    WgT = (np.asarray(W_g, np.float32) @ np.asarray(W_inp, np.float32)).T
    W3 = np.zeros((N, W3W), np.float32)
    W3[:, 0:GK] = W3f * SW3
    W3[:, GK:GK + G] = WgT * SW3
    W3 = np.ascontiguousarray(W3.astype(f8))
    bgp = (np.asarray(b_g, np.float32)
           + np.asarray(W_g, np.float32) @ np.asarray(b_inp, np.float32))
    binp = np.ascontiguousarray(np.asarray(b_inp, np.float32).reshape(1, W1C))
    crb = np.ones((128, 128), ml_dtypes.bfloat16)
    cf = np.zeros((128, 268), np.float32)
    cf[:, 0:D] = -np.asarray(centroids, np.float32)
    cf[:, 256:264] = np.exp(-bgp)[None, :]

    in_maps = []
    for b in range(8):
        Xb = np.ascontiguousarray((X[b] * SX).astype(f8))
        in_maps.append({"x": Xb, "w1": W1, "w3": W3,
                        "binp": binp, "crb": crb, "cf": cf})

    trace = os.environ.get("KERNEL_TRACE") == "1"
    r = run_bass_kernel_spmd(nc, in_maps, core_ids=list(range(8)), trace=trace)
    _cache["last_results"] = r
    out = np.empty((8, KC * D), np.float32)
    for b in range(8):
        vlad = r.results[b]["out"].astype(np.float64)       # [128, 256]
        nrm = np.sqrt((vlad * vlad).sum(axis=1, keepdims=True))
        out[b] = (vlad / (np.maximum(nrm, 1e-12) * np.sqrt(128.0))
                  ).reshape(-1).astype(np.float32)
    return out



# revision 11
# speedup vs baseline: 1.3629x; 1.3629x over previous
"""NextVLAD Trainium2 kernel v6 — 8-way data-parallel over batch (1 sample/core).

Host prep: x is token-normalized on host (exact), so every device scale is a
compile-time constant; W3 = [(W_gk@W_inp)^T | (W_g@W_inp)^T]*SW3 folds the
fc and gate projections for the softmax path; W1 = W_inp^T*SW1 feeds yT.
Final L2 norms and the +b_inp contribution to the VLAD sum are applied on
host (the device exports per-group sums Sg via one-hot columns in the vlad
matmul, so b_inp never has to be broadcast on-device).

v6 layout/schedule:
- every input host-packed to its [128, X] SBUF image; one contiguous DMA per
  tensor on the sync ring in strict priority order:
  x -> w3(blk0) -> w3(blk1,2) -> cf -> w1a -> w1b.
- gate logits folded into the z matmul: 1040-wide W3 (1024 gk + 8 gate + pad)
  processed as 3 col-blocks of 344 -> no separate tiny gate matmuls.
- yT = ry*YTS runs on the Act engine (Copy+scale from PSUM); wf = ex*sg*ise
  runs on DVE, all 32 tiles emitted right after the softmax denominators.

Per-core dataflow (sample b; M=512 tokens, N=1024, EN=2048, G=8, K=128, D=256):
  z[m,b344]  = x8^T W3-blk (fp8 DR)     ex = exp(z/16384)       bf16
  eg[m,g]    = exp(z_gate/16384) (f32)  sg = eg/(eg+e^{-bg'})   DVE
  se[blk]    = ones^T ex (bf16 mm)      ise = recip_approx(se)  f32
  rawY[m,e]  = x8^T W1-chunks (fp8 DR)  yT = rawY/128 (Act Copy) bf16
  wf[m,gk]   = ex * sg * ise (DVE)                              bf16
  vd[k,266]  = sum_{g,m} wf_g^T [yT_g | 1 1 | e_g]
               (col 256 = S[k], cols 258+g = Sg[k,g])
  out[k,:256]= vd - S*cent ; out[k,256:264] = Sg   (host: +Sg@binp, l2norm)
"""
import os
import numpy as np

N = 1024          # feature size
EN = 2048         # expanded features
G = 8             # groups
KC = 128          # clusters
D = 256           # per-group cluster dim
GK = G * KC       # 1024
BW = D + 10       # yT group block: 256 data | 2 ones | 8 one-hot = 266
W3W = 1040        # 1024 gk + 8 gates + 8 pad (row stride must be %16)
ZB = 344          # z col-block width (3 blocks cover 1032)
M = 512           # tokens per sample
MT = 4            # m-tiles of 128
SX = 8.0
SW1 = 16.0
SW3 = 2048.0
EXS = 1.0 / (SX * SW3)    # 1/16384
YTS = 1.0 / (SX * SW1)    # 1/128

_cache = {}


def _build_nc():
    import concourse.bacc as bacc
    import concourse.tile as tile
    from concourse import mybir

    f32 = mybir.dt.float32
    bf16 = mybir.dt.bfloat16
    fp8 = mybir.dt.float8e4
    Alu = mybir.AluOpType
    Act = mybir.ActivationFunctionType
    DR = mybir.MatmulPerfMode.DoubleRow

    nc = bacc.Bacc("TRN2", target_bir_lowering=False)
    # host-packed: row p, col (cs*W + j) holds source row cs*128+p, col j
    x0_d = nc.dram_tensor("x0", [128, 8 * 256], fp8, kind="ExternalInput")
    x1_d = nc.dram_tensor("x1", [128, 8 * 256], fp8, kind="ExternalInput")
    w3p0_d = nc.dram_tensor("w3p0", [128, 8 * ZB], fp8, kind="ExternalInput")
    w3p12_d = nc.dram_tensor("w3p12", [128, 8 * (W3W - ZB)], fp8,
                             kind="ExternalInput")
    w1a_d = nc.dram_tensor("w1a", [128, 8 * 1024], fp8, kind="ExternalInput")
    w1b_d = nc.dram_tensor("w1b", [128, 8 * 1024], fp8, kind="ExternalInput")
    cf_d = nc.dram_tensor("cf", [128, 268], f32, kind="ExternalInput")
    out_d = nc.dram_tensor("out", [KC, D + G], f32, kind="ExternalOutput")

    with tile.TileContext(nc) as tc:
        with tc.tile_pool(name="const", bufs=1) as constp, \
             tc.tile_pool(name="data", bufs=1) as datap, \
             tc.tile_pool(name="work", bufs=1) as workp, \
             tc.tile_pool(name="ps", bufs=1, space="PSUM") as ps:

            # ---------------- tiles ----------------
            cf_t = constp.tile([128, 268], f32)
            crb_t = constp.tile([128, 128], bf16)
            centn_t = cf_t[:, 0:D]
            ebg_t = cf_t[:, 256:264]
            xp_t = datap.tile([128, 8 * M], fp8)
            xv = xp_t.rearrange("p (cs m) -> p cs m", m=M)
            w3_t = datap.tile([128, 8 * W3W], fp8)
            w3v = w3_t.rearrange("p (cs j) -> p cs j", j=W3W)
            w1a_t = datap.tile([128, 8 * 1024], fp8)
            w1av = w1a_t.rearrange("p (cs j) -> p cs j", j=1024)
            w1b_t = datap.tile([128, 8 * 1024], fp8)
            w1bv = w1b_t.rearrange("p (cs j) -> p cs j", j=1024)

            # ------------- input DMA: one ring (sync), strict priority order ------
            nc.sync.dma_start(out=xv[:, :, 0:256], in_=x0_d[:])
            nc.sync.dma_start(out=w3v[:, :, 0:ZB], in_=w3p0_d[:])
            nc.sync.dma_start(out=xv[:, :, 256:512], in_=x1_d[:])
            nc.sync.dma_start(out=w3v[:, :, ZB:W3W], in_=w3p12_d[:])
            nc.sync.dma_start(out=cf_t[:], in_=cf_d[:])
            nc.sync.dma_start(out=w1a_t[:], in_=w1a_d[:])
            nc.sync.dma_start(out=w1b_t[:], in_=w1b_d[:])

            # persistent work tiles
            ex_t = [workp.tile([128, 3 * ZB], bf16, name=f"ex{m}")
                    for m in range(MT)]
            wf_t = [workp.tile([128, GK], bf16, name=f"wf{m}") for m in range(MT)]
            yT_t = [[workp.tile([128, 2 * BW], bf16, name=f"yT{e}_{m}")
                     for m in range(MT)] for e in range(4)]
            ise_t = workp.tile([128, 3 * ZB], f32)
            sg_t = workp.tile([128, 4 * G], f32)
            eg_t = workp.tile([128, 4 * G], f32)
            dum_t = workp.tile([128, 1], f32)
            dsrc_t = workp.tile([128, 1], f32)

            # warm the exp table early (no DMA dependency: memset source)
            nc.vector.memset(dsrc_t[:], 0.0)
            nc.scalar.activation(dum_t[:], dsrc_t[:], Act.Exp)
            # ones matrix for column sums: memset instead of DMA
            nc.vector.memset(crb_t[:], 1.0)

            # yT static columns: [256,257]=1 (S), [258+g]=1 only in group g's
            # block (exports Sg through the vlad matmul)
            for e in range(4):
                for m in range(MT):
                    eng = nc.vector if e < 2 else nc.gpsimd
                    yT3 = yT_t[e][m].rearrange("p (g c) -> p g c", c=BW)
                    eng.memset(yT3[:, :, D:D + 2], 1.0)
                    eng.memset(yT3[:, :, D + 2:BW], 0.0)
                    for blk in range(2):
                        g = 2 * e + blk
                        eng.memset(
                            yT_t[e][m][:, blk * BW + D + 2 + g:
                                       blk * BW + D + 3 + g], 1.0)

            # ---------------- phase A: gk+gate logits + exp ----------------
            # 3 col-blocks of 344 over the 1040-wide fused W3
            for blk in range(3):
                csl = slice(blk * ZB, (blk + 1) * ZB)
                for m in range(MT):
                    msl = slice(m * 128, (m + 1) * 128)
                    z_ps = ps.tile([128, 512], f32, name="z_ps", tag="zps",
                                   bufs=4)
                    for c in range(4):
                        nc.tensor.matmul(z_ps[:, 0:ZB],
                                         xv[:, 2 * c:2 * c + 2, msl],
                                         w3v[:, 2 * c:2 * c + 2, csl],
                                         start=(c == 0), stop=(c == 3),
                                         perf_mode=DR)
                    nc.scalar.activation(ex_t[m][:, csl], z_ps[:, 0:ZB],
                                         Act.Exp, scale=EXS)
                    if blk == 2:
                        # gate logits live in cols 1024:1032 = blk2 336:344
                        nc.scalar.activation(eg_t[:, m * G:(m + 1) * G],
                                             z_ps[:, ZB - 8:ZB],
                                             Act.Exp, scale=EXS)

            # ---------------- phase B: gates sg = eg/(eg+e^{-bg'}) (DVE) ----
            for m in range(MT):
                gs = slice(m * G, (m + 1) * G)
                wg = workp.tile([128, G], f32, name="wg", bufs=2)
                nc.vector.tensor_add(wg[:], eg_t[:, gs], ebg_t)
                rw = workp.tile([128, G], f32, name="rw", bufs=2)
                nc.vector.reciprocal(rw[:], wg[:])
                nc.vector.tensor_mul(sg_t[:, gs], eg_t[:, gs], rw[:])

            # ---------------- phase C: softmax denominators ----------------
            for blk in range(3):
                csl = slice(blk * ZB, (blk + 1) * ZB)
                se_ps = ps.tile([128, 512], f32, name="se_ps", tag="zps", bufs=4)
                for m in range(MT):
                    nc.tensor.matmul(se_ps[:, 0:ZB], crb_t[:],
                                     ex_t[m][:, csl],
                                     start=(m == 0), stop=(m == MT - 1))
                nc.vector.reciprocal_approx_fast(
                    out=ise_t[:, csl], in_=se_ps[:, 0:ZB])

            # ---------------- wf = ex*sg*ise, all tiles up front (DVE) ------
            for g in range(G):
                gsl = slice(g * KC, (g + 1) * KC)
                for m in range(MT):
                    nc.vector.scalar_tensor_tensor(
                        out=wf_t[m][:, gsl], in0=ex_t[m][:, gsl],
                        scalar=sg_t[:, m * G + g:m * G + g + 1],
                        in1=ise_t[:, gsl], op0=Alu.mult, op1=Alu.mult)

            # ---------------- phase D: yT on Act, vlad trailing -------------
            vd_ps = ps.tile([128, 512], f32, name="vd_ps", tag="gps", bufs=1)
            for ec in range(4):
                for m in range(MT):
                    msl = slice(m * 128, (m + 1) * 128)
                    ry_ps = ps.tile([128, 512], f32, name="ry_ps", tag="yps",
                                    bufs=3)
                    w1hv = (w1av, w1bv)[ec // 2]
                    ecs = (ec % 2) * 512
                    for c in range(4):
                        nc.tensor.matmul(ry_ps[:], xv[:, 2 * c:2 * c + 2, msl],
                                         w1hv[:, 2 * c:2 * c + 2, ecs:ecs + 512],
                                         start=(c == 0), stop=(c == 3),
                                         perf_mode=DR)
                    yT3 = yT_t[ec][m].rearrange("p (g c) -> p g c", c=BW)
                    nc.scalar.activation(
                        yT3[:, 0:2, 0:D],
                        ry_ps[:].rearrange("p (g c) -> p g c", c=D),
                        Act.Copy, scale=YTS)
                # vlad groups one ec late (their wf/yT are complete by now)
                if ec >= 1:
                    for g in (2 * ec - 2, 2 * ec - 1):
                        for m in range(MT):
                            nc.tensor.matmul(
                                vd_ps[:, 0:BW], wf_t[m][:, g * KC:(g + 1) * KC],
                                yT_t[g // 2][m][:, (g % 2) * BW:(g % 2 + 1) * BW],
                                start=(g == 0 and m == 0), stop=False)

            # ---------------- phase E: remaining vlad + centroid + out ------
            for g in (6, 7):
                for m in range(MT):
                    nc.tensor.matmul(vd_ps[:, 0:BW],
                                     wf_t[m][:, g * KC:(g + 1) * KC],
                                     yT_t[g // 2][m][:, (g % 2) * BW:(g % 2 + 1) * BW],
                                     start=False, stop=(g == 7 and m == MT - 1))
            vlad_t = workp.tile([128, D + G], f32)
            nc.vector.scalar_tensor_tensor(
                out=vlad_t[:, 0:D], in0=centn_t[:], scalar=vd_ps[:, D:D + 1],
                in1=vd_ps[:, 0:D], op0=Alu.mult, op1=Alu.add)
            nc.scalar.activation(vlad_t[:, D:D + G], vd_ps[:, D + 2:D + 2 + G],
                                 Act.Copy)
            nc.sync.dma_start(out=out_d[0:64, :], in_=vlad_t[0:64, :])
            nc.scalar.dma_start(out=out_d[64:128, :], in_=vlad_t[64:128, :])

    nc.compile()
    return nc


def _get_nc():
    if "nc" not in _cache:
        _cache["nc"] = _build_nc()
    return _cache["nc"]


def _pack(a):
    """[1024, C] -> [128, 8*C]: row p col (cs*C+j) = a[cs*128+p, j]."""
    c = a.shape[1]
    return np.ascontiguousarray(
        a.reshape(8, 128, c).transpose(1, 0, 2).reshape(128, 8 * c))


def kernel(x, W_inp, b_inp, W_g, b_g, W_gk, b_gk, centroids):
    from concourse.bass_utils import run_bass_kernel_spmd
    import ml_dtypes
    f8 = ml_dtypes.float8_e4m3

    nc = _get_nc()

    x = np.asarray(x, dtype=np.float32)
    X = x.reshape(8, 8, N, 64).transpose(0, 2, 1, 3).reshape(8, N, M)
    # exact per-token normalization on host
    X = X / np.maximum(np.linalg.norm(X, axis=1, keepdims=True), 1e-12)
    W1 = (np.asarray(W_inp, np.float32).T * SW1).astype(f8)
    W1a = _pack(W1[:, 0:1024])
    W1b = _pack(W1[:, 1024:2048])
    W3f = (np.asarray(W_gk, np.float32) @ np.asarray(W_inp, np.float32)).T
    WgT = (np.asarray(W_g, np.float32) @ np.asarray(W_inp, np.float32)).T
    W3 = np.zeros((N, W3W), np.float32)
    W3[:, 0:GK] = W3f * SW3
    W3[:, GK:GK + G] = WgT * SW3
    W3 = W3.astype(f8)
    W3p0 = _pack(W3[:, 0:ZB])
    W3p12 = _pack(W3[:, ZB:W3W])
    bgp = (np.asarray(b_g, np.float32)
           + np.asarray(W_g, np.float32) @ np.asarray(b_inp, np.float32))
    cf = np.zeros((128, 268), np.float32)
    cf[:, 0:D] = -np.asarray(centroids, np.float32)
    cf[:, 256:264] = np.exp(-bgp)[None, :]

    in_maps = []
    for b in range(8):
        Xs = (X[b] * SX).astype(f8)
        in_maps.append({"x0": _pack(Xs[:, 0:256]), "x1": _pack(Xs[:, 256:512]),
                        "w3p0": W3p0, "w3p12": W3p12,
                        "w1a": W1a, "w1b": W1b, "cf": cf})

    trace = os.environ.get("KERNEL_TRACE") == "1"
    r = run_bass_kernel_spmd(nc, in_maps, core_ids=list(range(8)), trace=trace)
    _cache["last_results"] = r
    binp_r = np.asarray(b_inp, np.float64).reshape(G, D)
    out = np.empty((8, KC * D), np.float32)
    for b in range(8):
        raw = r.results[b]["out"].astype(np.float64)        # [128, 264]
        # add back the Sg @ b_inp contribution (device computed y w/o b_inp)
        vlad = raw[:, 0:D] + raw[:, D:D + G] @ binp_r
        nrm = np.sqrt((vlad * vlad).sum(axis=1, keepdims=True))
        out[b] = (vlad / (np.maximum(nrm, 1e-12) * np.sqrt(128.0))
                  ).reshape(-1).astype(np.float32)
    return out
